# revision 18
# baseline (speedup 1.0000x reference)
"""Trainium2 Bass kernel for nn_FDC2_61108794688088.

Math: out[i, c] = BS * s1[i, c] + (W2 @ colsum)[c] + BS * b_fc[c]
  where s1 = z1 @ W_fc[:, :2048].T
        colsum = sum_j relu(z2f @ W_proj.T + b_proj)[j, :]
        W2 = W_fc[:, 2048:]

Sharding: data-parallel over batch across 8 cores. Each core computes
  - s1T_scaled = (BS * s1_shard).T            [65, 256]  (float32r matmul)
  - colsum_local [1024] of its 256 rows       (fp8 matmul, fp32 accum)
The only cross-core reduction is the [1024] colsum vector, done on host
during the gather step, along with the tiny [65] matvec against W2.

The projection matmul runs in fp8 E4M3 (weights pre-scaled by 64 so they
sit in the normal range; the 1/64 is folded into the relu's scale) with
DoubleRow packing: K is consumed 256 rows per matmul instruction.
"""

import os
import sys

import numpy as np


def _import_concourse():
    try:
        import concourse.bass  # noqa: F401
    except ImportError:
        for p in ("/opt/trn_rl_repo", "/root/.axon_site/_ro/trn_rl_repo"):
            if os.path.isdir(p) and p not in sys.path:
                sys.path.append(p)
        import concourse.bass  # noqa: F401


_import_concourse()

import ml_dtypes  # noqa: E402
from contextlib import ExitStack  # noqa: E402

import concourse.bacc as bacc  # noqa: E402
import concourse.tile as tile  # noqa: E402
from concourse.tile_rust import add_dep_helper  # noqa: E402
from concourse import mybir  # noqa: E402
from concourse import bass_utils  # noqa: E402

BS = 2048
HID = 2048
PIN = 3 * 56 * 56  # 9408
POUT = 1024
NCLS = 65
NCORES = 8
B = BS // NCORES  # 256 rows per core
KT2 = (PIN + 127) // 128  # 74 k-tiles for the projection (padded to 9472)
KP2 = KT2 // 2  # 37 DoubleRow k-pairs
KT1 = HID // 128  # 16 k-tiles for s1
MT = POUT // 128  # 8 m-tiles of output features
WSCALE = 64.0  # fp8 weight pre-scale

FP8 = ml_dtypes.float8_e4m3
N_Z2_CHUNKS = 4  # split the z2 load so PE can start early
N_WP_CHUNKS = 2

_NC_CACHE = None
LAST_RESULTS = None  # BassKernelResults of the most recent run (for profiling)


def _chunks(n, k):
    base, rem = divmod(n, k)
    out = []
    start = 0
    for i in range(k):
        size = base + (1 if i < rem else 0)
        out.append((start, size))
        start += size
    return out


def _build_nc():
    """Build the per-core Bass module (identical on all 8 cores)."""
    # Bacc (not raw Bass): its compile passes split multi-semaphore waits
    # into EventSemaphore instructions (TRN2 allows 1 wait per instruction).
    nc = bacc.Bacc(target_bir_lowering=False)
    dt = mybir.dt

    z2ft = nc.dram_tensor("z2ft", [128, KP2, 2, B], dt.float8e4, kind="ExternalInput")
    wpt = nc.dram_tensor(
        "wpt", [MT, 128, KP2, 2, 128], dt.float8e4, kind="ExternalInput"
    )
    bp = nc.dram_tensor("bp", [128, MT], dt.float32, kind="ExternalInput")
    # z1^T shard and 2048*W_fc[:, :2048]^T fused into one tensor so the first
    # float32r matmul (self-loading, single sync-wait slot) waits on one DMA.
    zw = nc.dram_tensor("zw", [128, KT1, B + NCLS], dt.float32r, kind="ExternalInput")

    s1t_out = nc.dram_tensor("s1t", [NCLS, B], dt.float32, kind="ExternalOutput")
    colsum_out = nc.dram_tensor("colsum", [128, MT], dt.float32, kind="ExternalOutput")

    with tile.TileContext(nc) as tc, ExitStack() as ctx:
        singles = ctx.enter_context(tc.tile_pool(name="singles", bufs=1))
        wp_pool = ctx.enter_context(tc.tile_pool(name="wp", bufs=3))
        ps_pool = ctx.enter_context(tc.tile_pool(name="ps", bufs=2, space="PSUM"))
        ps1_pool = ctx.enter_context(tc.tile_pool(name="ps1", bufs=1, space="PSUM"))
        relu_pool = ctx.enter_context(tc.tile_pool(name="relu", bufs=2))
        out_pool = ctx.enter_context(tc.tile_pool(name="outs", bufs=1))

        # One HWDGE queue (aggregate bandwidth is highest single-queue, and
        # completion within a queue is FIFO): issue in exact consumption
        # order — z2 head chunk, wp0, rest of z2, then the wp stream with zw
        # slotted in before the s1 anchor point.
        z2_sb = singles.tile([128, KP2, 2, B], dt.float8e4)
        zw_sb = singles.tile([128, KT1, B + NCLS], dt.float32r)
        bp_sb = singles.tile([128, MT], dt.float32)

        nc.sync.dma_start(out=z2_sb[:, 0:4], in_=z2ft[:, 0:4])
        nc.sync.dma_start(out=bp_sb, in_=bp[:])

        colsum_sb = out_pool.tile([128, MT], dt.float32)

        # projection branch: for each 128-wide block of output features,
        # psum[m, n] = sum_K 64*W_proj[m, K] * z2f[n, K]  (DoubleRow fp8),
        # then relu(psum/64 + b) and row-sum over the local batch.
        proj_mms = []
        for t in range(MT):
            wp_sb = wp_pool.tile([128, KP2, 2, 128], dt.float8e4, tag="wp")
            if t == 0:
                nc.sync.dma_start(out=wp_sb[:, 0:4], in_=wpt[t, :, 0:4])
                nc.sync.dma_start(out=wp_sb[:, 4:KP2], in_=wpt[t, :, 4:KP2])
                # remaining z2 right after wp0 (every m-tile needs all of z2)
                nc.sync.dma_start(out=z2_sb[:, 4:20], in_=z2ft[:, 4:20])
                nc.sync.dma_start(out=z2_sb[:, 20:KP2], in_=z2ft[:, 20:KP2])
            else:
                nc.sync.dma_start(out=wp_sb, in_=wpt[t])
                if t == 1:
                    # zw follows wp1: it is only needed at the s1 slot after
                    # m-tile 3, by which point it has long arrived.
                    nc.sync.dma_start(out=zw_sb, in_=zw[:])
            ps = ps_pool.tile([128, B], dt.float32, tag="ps")
            for kp in range(KP2):
                mm = nc.tensor.matmul(
                    ps,
                    lhsT=wp_sb[:, kp],
                    rhs=z2_sb[:, kp],
                    start=(kp == 0),
                    stop=(kp == KP2 - 1),
                    perf_mode=mybir.MatmulPerfMode.DoubleRow,
                )
                proj_mms.append(mm)
            relu_sb = relu_pool.tile([128, B], dt.float32, tag="relu")
            nc.scalar.activation(
                out=relu_sb,
                in_=ps,
                func=mybir.ActivationFunctionType.Relu,
                bias=bp_sb[:, t : t + 1],
                scale=1.0 / WSCALE,
                accum_out=colsum_sb[:, t : t + 1],
            )
        nc.sync.dma_start(out=colsum_out[:], in_=colsum_sb)

        # s1 branch: psum[c, n] = sum_K 2048*W_fc[c, K] * z1[n, K] (K-tiled).
        # The PE stream is in-order; slot these after m-tile 3 — by then the
        # zw DMA has drained, and the PE is starving for wp anyway.
        anchor_mm = proj_mms[4 * KP2 - 1]
        ps1 = ps1_pool.tile([NCLS, B], dt.float32, tag="ps1")
        for ki in range(KT1):
            mm = nc.tensor.matmul(
                ps1,
                lhsT=zw_sb[:, ki, B:],
                rhs=zw_sb[:, ki, :B],
                start=(ki == 0),
                stop=(ki == KT1 - 1),
            )
            if ki == 0:
                add_dep_helper(
                    mm.ins, anchor_mm.ins, reason="s1 after projection m-tile 3"
                )
        add_dep_helper(
            proj_mms[4 * KP2].ins, mm.ins, reason="m-tile 4 after s1"
        )
        s1_sb = out_pool.tile([NCLS, B], dt.float32)
        nc.vector.tensor_copy(out=s1_sb, in_=ps1)
        nc.scalar.dma_start(out=s1t_out[:], in_=s1_sb)

    if not nc.is_finalized():
        nc.finalize()
    return nc


def _prep_inputs(z1, z2, W_proj, b_proj, W_fc):
    """Host-side sharding + layout. Returns per-core input maps."""
    z2f = np.ascontiguousarray(z2.reshape(BS, PIN))

    # z2f^T padded to [74*128, 2048] fp8, per-core [128, 37, 2, 256]:
    # z2ft[p, t, j, n] = z2f^T[(2t+j)*128 + p, n]
    Z = np.zeros((KT2 * 128, BS), dtype=FP8)
    Z[:PIN] = z2f.T.astype(FP8)

    # 64 * W_proj^T padded, arranged [8, 128, 37, 2, 128]:
    # wpt[t, p, k, j, m] = 64*W_proj[t*128+m, (2k+j)*128+p]
    Wp = np.zeros((KT2 * 128, POUT), dtype=FP8)
    Wp[:PIN] = (W_proj.T * np.float32(WSCALE)).astype(FP8)
    wpt_host = np.ascontiguousarray(
        Wp.reshape(KP2, 2, 128, MT, 128).transpose(3, 2, 0, 1, 4)
    )

    bp_host = np.ascontiguousarray(b_proj.reshape(MT, 128).T).astype(np.float32)

    # 2048 * W_fc[:, :HID]^T arranged [128, 16, 65]
    w1t_host = np.ascontiguousarray(
        (np.float32(BS) * W_fc[:, :HID].T.astype(np.float32))
        .reshape(KT1, 128, NCLS)
        .transpose(1, 0, 2)
    ).astype(np.float32)

    in_maps = []
    for c in range(NCORES):
        sl = slice(c * B, (c + 1) * B)
        z2_shard = np.ascontiguousarray(
            Z[:, sl].reshape(KP2, 2, 128, B).transpose(2, 0, 1, 3)
        )
        z1_shard = (
            z1[sl].T.reshape(KT1, 128, B).transpose(1, 0, 2).astype(np.float32)
        )
        zw_shard = np.ascontiguousarray(
            np.concatenate([z1_shard, w1t_host], axis=2)
        )
        in_maps.append(
            {
                "z2ft": z2_shard,
                "wpt": wpt_host,
                "bp": bp_host,
                "zw": zw_shard,
            }
        )
    return in_maps


def kernel(z1, z2, W_proj, b_proj, W_fc, b_fc):
    global _NC_CACHE, LAST_RESULTS

    z1 = np.asarray(z1, dtype=np.float32)
    z2 = np.asarray(z2, dtype=np.float32)
    W_proj = np.asarray(W_proj, dtype=np.float32)
    b_proj = np.asarray(b_proj, dtype=np.float32)
    W_fc = np.asarray(W_fc, dtype=np.float32)
    b_fc = np.asarray(b_fc, dtype=np.float32)

    if _NC_CACHE is None:
        _NC_CACHE = _build_nc()
    nc = _NC_CACHE

    in_maps = _prep_inputs(z1, z2, W_proj, b_proj, W_fc)
    res = bass_utils.run_bass_kernel_spmd(nc, in_maps, core_ids=list(range(NCORES)))
    LAST_RESULTS = res

    # gather: concat s1T shards, sum colsum shards, add the broadcast vector
    A = np.concatenate([np.asarray(r["s1t"]).T for r in res.results], axis=0)
    colsum = np.zeros(POUT, dtype=np.float64)
    for r in res.results:
        colsum += np.asarray(r["colsum"]).T.reshape(POUT).astype(np.float64)
    vec = W_fc[:, HID:].astype(np.float64) @ colsum + np.float64(BS) * b_fc.astype(
        np.float64
    )
    out = A.astype(np.float64) + vec[None, :]
    return out.astype(np.float32)


# revision 20
# speedup vs baseline: 1.1193x; 1.1193x over previous
"""Trainium2 Bass kernel for nn_FDC2_61108794688088.

Math: out[i, c] = BS * s1[i, c] + (W2 @ colsum)[c] + BS * b_fc[c]
  where s1 = z1 @ W_fc[:, :2048].T
        colsum = sum_j relu(z2f @ W_proj.T + b_proj)[j, :]
        W2 = W_fc[:, 2048:]

Sharding: data-parallel over batch across 8 cores. Each core computes
  - s1T_scaled = (BS * s1_shard).T            [65, 256]  (float32r matmul)
  - colsum_local [1024] of its 256 rows       (fp8 matmul, fp32 accum)
The only cross-core reduction is the [1024] colsum vector, done on host
during the gather step, along with the tiny [65] matvec against W2.

The projection matmul runs in fp8 E4M3 (weights pre-scaled by 64 so they
sit in the normal range; the 1/64 is folded into the relu's scale) with
DoubleRow packing: K is consumed 256 rows per matmul instruction.
"""

import os
import sys

import numpy as np


def _import_concourse():
    try:
        import concourse.bass  # noqa: F401
    except ImportError:
        for p in ("/opt/trn_rl_repo", "/root/.axon_site/_ro/trn_rl_repo"):
            if os.path.isdir(p) and p not in sys.path:
                sys.path.append(p)
        import concourse.bass  # noqa: F401


_import_concourse()

import ml_dtypes  # noqa: E402
from contextlib import ExitStack  # noqa: E402

import concourse.bacc as bacc  # noqa: E402
import concourse.tile as tile  # noqa: E402
from concourse.tile_rust import add_dep_helper  # noqa: E402
from concourse import mybir  # noqa: E402
from concourse import bass_utils  # noqa: E402

BS = 2048
HID = 2048
PIN = 3 * 56 * 56  # 9408
POUT = 1024
NCLS = 65
NCORES = 8
B = BS // NCORES  # 256 rows per core
KT2 = (PIN + 127) // 128  # 74 k-tiles for the projection (padded to 9472)
KP2 = KT2 // 2  # 37 DoubleRow k-pairs
KT1 = HID // 128  # 16 k-tiles for s1
MT = POUT // 128  # 8 m-tiles of output features
WSCALE = 64.0  # fp8 weight pre-scale

FP8 = ml_dtypes.float8_e4m3
N_Z2_CHUNKS = 4  # split the z2 load so PE can start early
N_WP_CHUNKS = 2

_NC_CACHE = None
LAST_RESULTS = None  # BassKernelResults of the most recent run (for profiling)


def _chunks(n, k):
    base, rem = divmod(n, k)
    out = []
    start = 0
    for i in range(k):
        size = base + (1 if i < rem else 0)
        out.append((start, size))
        start += size
    return out


def _build_nc():
    """Build the per-core Bass module (identical on all 8 cores).

    Raw Bacc (no TileContext): everything stays resident in SBUF (no pool
    recycling, so no WAR hazards), PSUM gives each m-tile its own bank, and
    ordering is a handful of hand-placed semaphores. This skips Tile's
    multi-microsecond entry/exit barriers and ~190-semaphore teardown.
    """
    nc = bacc.Bacc(target_bir_lowering=False)
    dt = mybir.dt

    z2ft = nc.dram_tensor("z2ft", [128, KP2, 2, B], dt.float8e4, kind="ExternalInput")
    wpt = nc.dram_tensor(
        "wpt", [MT, 128, KP2, 2, 128], dt.float8e4, kind="ExternalInput"
    )
    bp = nc.dram_tensor("bp", [128, MT], dt.float32, kind="ExternalInput")
    # z1^T shard and 2048*W_fc[:, :2048]^T fused into one tensor so the first
    # float32r matmul (self-loading, single sync-wait slot) waits on one DMA.
    zw = nc.dram_tensor("zw", [128, KT1, B + NCLS], dt.float32r, kind="ExternalInput")

    s1t_out = nc.dram_tensor("s1t", [NCLS, B], dt.float32, kind="ExternalOutput")
    colsum_out = nc.dram_tensor("colsum", [128, MT], dt.float32, kind="ExternalOutput")

    # SBUF: everything resident simultaneously (~116 KB/partition of 192).
    z2_sb = nc.alloc_sbuf_tensor("z2_sb", [128, KP2, 2, B], dt.float8e4)[:]
    zw_sb = nc.alloc_sbuf_tensor("zw_sb", [128, KT1, B + NCLS], dt.float32r)[:]
    bp_sb = nc.alloc_sbuf_tensor("bp_sb", [128, MT], dt.float32)[:]
    wp_sb = [
        nc.alloc_sbuf_tensor(f"wp_sb{t}", [128, KP2, 2, 128], dt.float8e4)[:]
        for t in range(MT)
    ]
    relu_sb = nc.alloc_sbuf_tensor("relu_sb", [128, B], dt.float32)[:]
    colsum_sb = nc.alloc_sbuf_tensor("colsum_sb", [128, MT], dt.float32)[:]
    s1_sb = nc.alloc_sbuf_tensor("s1_sb", [NCLS, B], dt.float32)[:]

    # PSUM: one bank per m-tile; s1 reuses bank 0 after act0 consumed it.
    ps = [
        nc.alloc_psum_tensor(f"ps{t}", [128, B], dt.float32)[:] for t in range(MT)
    ]
    ps1 = ps[0][:NCLS, :]

    # Semaphores: one per input DMA (sync-engine DMAs fan out over several
    # HW queues, so cumulative FIFO thresholds on a shared sem are unsafe).
    s_z2a = nc.alloc_semaphore("s_z2a")
    s_z2b = nc.alloc_semaphore("s_z2b")
    s_bp = nc.alloc_semaphore("s_bp")
    s_zw = nc.alloc_semaphore("s_zw")
    s_wp = [nc.alloc_semaphore(f"s_wp{t}") for t in range(MT)]
    pesem = nc.alloc_semaphore("pesem")  # +1 per finished psum group
    actsem = nc.alloc_semaphore("actsem")  # +1 per finished activation
    vsem = nc.alloc_semaphore("vsem")  # s1 psum->sbuf copy done
    qout1 = nc.alloc_semaphore("qout1")  # s1t output DMA
    qout2 = nc.alloc_semaphore("qout2")  # colsum output DMA
    donesem = nc.alloc_semaphore("donesem")
    all_sems = (
        [s_z2a, s_z2b, s_bp, s_zw]
        + s_wp
        + [pesem, actsem, vsem, qout1, qout2, donesem]
    )

    with nc.Block() as block:

        @block.sync
        def _(sync):
            # issue order approximates stream priority
            sync.dma_start(out=z2_sb[:, 0:4], in_=z2ft[:, 0:4]).then_inc(s_z2a, 16)
            sync.dma_start(out=wp_sb[0][:, 0:4], in_=wpt[0, :, 0:4]).then_inc(
                s_wp[0], 16
            )
            sync.dma_start(out=z2_sb[:, 4:KP2], in_=z2ft[:, 4:KP2]).then_inc(
                s_z2b, 16
            )
            sync.dma_start(out=wp_sb[0][:, 4:KP2], in_=wpt[0, :, 4:KP2]).then_inc(
                s_wp[0], 16
            )
            sync.dma_start(out=bp_sb, in_=bp[:]).then_inc(s_bp, 16)
            sync.dma_start(out=wp_sb[1], in_=wpt[1]).then_inc(s_wp[1], 16)
            sync.dma_start(out=zw_sb, in_=zw[:]).then_inc(s_zw, 16)
            for t in range(2, MT):
                sync.dma_start(out=wp_sb[t], in_=wpt[t]).then_inc(s_wp[t], 16)
            # s1 output after the vector copy
            sync.wait_ge(vsem, 1)
            sync.dma_start(out=s1t_out[:], in_=s1_sb).then_inc(qout1, 16)
            sync.wait_ge(qout1, 16)
            sync.sem_inc(donesem, 1)

        @block.tensor
        def _(tensor):
            def proj_tile(t, seg_waits):
                for kp in range(KP2):
                    if kp in seg_waits:
                        for sem, val in seg_waits[kp]:
                            tensor.wait_ge(sem, val)
                    mm = nc.tensor.matmul(
                        ps[t],
                        lhsT=wp_sb[t][:, kp],
                        rhs=z2_sb[:, kp],
                        start=(kp == 0),
                        stop=(kp == KP2 - 1),
                        perf_mode=mybir.MatmulPerfMode.DoubleRow,
                    )
                mm.then_inc(pesem, 1)

            proj_tile(
                0,
                {
                    0: [(s_z2a, 16), (s_wp[0], 16)],
                    4: [(s_z2b, 16), (s_wp[0], 32)],
                },
            )
            for t in range(1, 4):
                proj_tile(t, {0: [(s_wp[t], 16)]})
            # s1 slot: zw has arrived by now; bank-0 psum is free once act0
            # ran. 16 float32r matmuls accumulate 2048*s1^T.
            tensor.wait_ge(s_zw, 16)
            tensor.wait_ge(actsem, 1)
            for ki in range(KT1):
                mm = nc.tensor.matmul(
                    ps1,
                    lhsT=zw_sb[:, ki, B:],
                    rhs=zw_sb[:, ki, :B],
                    start=(ki == 0),
                    stop=(ki == KT1 - 1),
                )
            mm.then_inc(pesem, 1)
            for t in range(4, MT):
                proj_tile(t, {0: [(s_wp[t], 16)]})

        @block.scalar
        def _(scalar):
            scalar.wait_ge(s_bp, 16)
            # pesem counts: m0..m3 -> 1..4, s1 -> 5, m4..m7 -> 6..9
            thresholds = [1, 2, 3, 4, 6, 7, 8, 9]
            for t in range(MT):
                scalar.wait_ge(pesem, thresholds[t])
                nc.scalar.activation(
                    out=relu_sb,
                    in_=ps[t],
                    func=mybir.ActivationFunctionType.Relu,
                    bias=bp_sb[:, t : t + 1],
                    scale=1.0 / WSCALE,
                    accum_out=colsum_sb[:, t : t + 1],
                ).then_inc(actsem, 1)
            nc.scalar.dma_start(out=colsum_out[:], in_=colsum_sb).then_inc(qout2, 16)
            scalar.wait_ge(qout2, 16)
            scalar.sem_inc(donesem, 1)

        @block.vector
        def _(vector):
            vector.wait_ge(pesem, 5)
            nc.vector.tensor_copy(out=s1_sb, in_=ps1).then_inc(vsem, 1)

        @block.gpsimd
        def _(gpsimd):
            gpsimd.wait_ge(donesem, 2)
            for sem in all_sems:
                gpsimd.sem_clear(sem)

    if not nc.is_finalized():
        nc.finalize()
    return nc


def _prep_inputs(z1, z2, W_proj, b_proj, W_fc):
    """Host-side sharding + layout. Returns per-core input maps."""
    z2f = np.ascontiguousarray(z2.reshape(BS, PIN))

    # z2f^T padded to [74*128, 2048] fp8, per-core [128, 37, 2, 256]:
    # z2ft[p, t, j, n] = z2f^T[(2t+j)*128 + p, n]
    Z = np.zeros((KT2 * 128, BS), dtype=FP8)
    Z[:PIN] = z2f.T.astype(FP8)

    # 64 * W_proj^T padded, arranged [8, 128, 37, 2, 128]:
    # wpt[t, p, k, j, m] = 64*W_proj[t*128+m, (2k+j)*128+p]
    Wp = np.zeros((KT2 * 128, POUT), dtype=FP8)
    Wp[:PIN] = (W_proj.T * np.float32(WSCALE)).astype(FP8)
    wpt_host = np.ascontiguousarray(
        Wp.reshape(KP2, 2, 128, MT, 128).transpose(3, 2, 0, 1, 4)
    )

    bp_host = np.ascontiguousarray(b_proj.reshape(MT, 128).T).astype(np.float32)

    # 2048 * W_fc[:, :HID]^T arranged [128, 16, 65]
    w1t_host = np.ascontiguousarray(
        (np.float32(BS) * W_fc[:, :HID].T.astype(np.float32))
        .reshape(KT1, 128, NCLS)
        .transpose(1, 0, 2)
    ).astype(np.float32)

    in_maps = []
    for c in range(NCORES):
        sl = slice(c * B, (c + 1) * B)
        z2_shard = np.ascontiguousarray(
            Z[:, sl].reshape(KP2, 2, 128, B).transpose(2, 0, 1, 3)
        )
        z1_shard = (
            z1[sl].T.reshape(KT1, 128, B).transpose(1, 0, 2).astype(np.float32)
        )
        zw_shard = np.ascontiguousarray(
            np.concatenate([z1_shard, w1t_host], axis=2)
        )
        in_maps.append(
            {
                "z2ft": z2_shard,
                "wpt": wpt_host,
                "bp": bp_host,
                "zw": zw_shard,
            }
        )
    return in_maps


def kernel(z1, z2, W_proj, b_proj, W_fc, b_fc):
    global _NC_CACHE, LAST_RESULTS

    z1 = np.asarray(z1, dtype=np.float32)
    z2 = np.asarray(z2, dtype=np.float32)
    W_proj = np.asarray(W_proj, dtype=np.float32)
    b_proj = np.asarray(b_proj, dtype=np.float32)
    W_fc = np.asarray(W_fc, dtype=np.float32)
    b_fc = np.asarray(b_fc, dtype=np.float32)

    if _NC_CACHE is None:
        _NC_CACHE = _build_nc()
    nc = _NC_CACHE

    in_maps = _prep_inputs(z1, z2, W_proj, b_proj, W_fc)
    res = bass_utils.run_bass_kernel_spmd(nc, in_maps, core_ids=list(range(NCORES)))
    LAST_RESULTS = res

    # gather: concat s1T shards, sum colsum shards, add the broadcast vector
    A = np.concatenate([np.asarray(r["s1t"]).T for r in res.results], axis=0)
    colsum = np.zeros(POUT, dtype=np.float64)
    for r in res.results:
        colsum += np.asarray(r["colsum"]).T.reshape(POUT).astype(np.float64)
    vec = W_fc[:, HID:].astype(np.float64) @ colsum + np.float64(BS) * b_fc.astype(
        np.float64
    )
    out = A.astype(np.float64) + vec[None, :]
    return out.astype(np.float32)
